# revision 10
# baseline (speedup 1.0000x reference)
"""Trainium2 Bass kernel for quantized InvertedResidual block (DoReFa fake-quant).

Strategy:
- Data-parallel: 32 images -> 4 per core across 8 NeuronCores.
- Host precomputes DoReFa-quantized weights, folds BN scale (and the
  42.5 / 255 quant scales) into matmul weights, and appends bias +
  2^23 magic-rounding rows to the conv1 weight matrix.
- Device per image:
  stage1: 1x1 conv (K=66 incl. bias+magic rows) -> PSUM holds
          round_int(bn(y)*42.5)+2^23; ACT Relu(x-2^23) -> integers,
          DVE min 255 -> r1 in 0..255 stored f32 in a zero-padded
          [128, 58*58] SBUF layout per 128-channel group.
  stage2: depthwise 3x3 = 9 diagonal matmuls (weights w2q*inv2*42.5)
          accumulating in PSUM over shifted 2D access patterns;
          ACT Identity(+beta2*42.5), DVE (add 2^23, max 2^23),
          DVE (min 2^23+255, sub 2^23) -> r2 integers.
  stage3: 1x1 conv over r2 (weights w3q*6*inv3), ACT Identity(+beta3*255),
          two DVE clamps, then (r3/255) + x residual via scalar_tensor_tensor.
All arithmetic fp32; activations kept on the exact 8-bit integer grid.
"""
import numpy as np

EPS = 1e-5
MAGIC = np.float32(2.0 ** 23)

B, C, H, W = 32, 64, 56, 56
HID = 384
NCORES = 8
BPC = B // NCORES          # images per core
PIX = H * W                # 3136
PW = W + 2                 # 58
PH = H + 2
PPIX = PW * PH             # 3364
NT = 7                     # pixel tiles per image
TW = PIX // NT             # 448 = 8 rows x 56
ROWS_PT = H // NT          # 8
NG = HID // 128            # 3 channel groups

_cache = {}


def _quant_w(w):
    # DoReFa weight fake-quant, computed with jax on CPU so tanh/round are
    # bitwise identical to the reference implementation.
    import jax
    import jax.numpy as jnp
    with jax.default_device(jax.devices('cpu')[0]):
        t = jnp.tanh(jnp.asarray(w, jnp.float32))
        m = jnp.max(jnp.abs(t), axis=(1, 2, 3), keepdims=True)
        wn = t / (2.0 * m) + 0.5
        q = 2.0 * jnp.round(wn * 255.0) / 255.0 - 1.0
        return np.asarray(q, np.float32)


def _build_program():
    import concourse.bass as bass
    import concourse.tile as tile
    from concourse import bacc, mybir

    fp32 = mybir.dt.float32
    nc = bacc.Bacc("TRN2", target_bir_lowering=False, debug=False,
                   enable_asserts=False, num_devices=NCORES)

    xs = nc.dram_tensor("xs", [BPC, 66, PIX], fp32, kind="ExternalInput").ap()
    w1f = nc.dram_tensor("w1f", [66, HID], fp32, kind="ExternalInput").ap()
    wdw = nc.dram_tensor("wdw", [128, NG * 9 * 128], fp32, kind="ExternalInput").ap()
    b2m = nc.dram_tensor("b2m", [128, NG], fp32, kind="ExternalInput").ap()
    w3f = nc.dram_tensor("w3f", [128, NG * 64], fp32, kind="ExternalInput").ap()
    b3m = nc.dram_tensor("b3m", [64, 1], fp32, kind="ExternalInput").ap()
    ys = nc.dram_tensor("ys", [BPC, 64, PIX], fp32, kind="ExternalOutput").ap()

    mx = mybir.AluOpType.max
    mn = mybir.AluOpType.min
    add = mybir.AluOpType.add
    sub = mybir.AluOpType.subtract
    mult = mybir.AluOpType.mult
    RELU = mybir.ActivationFunctionType.Relu  # noqa: F841
    COPY = mybir.ActivationFunctionType.Copy
    IDENT = mybir.ActivationFunctionType.Identity

    with tile.TileContext(nc) as tc:
        from contextlib import ExitStack
        with ExitStack() as ctx:
            consts = ctx.enter_context(tc.tile_pool(name="consts", bufs=1))
            h1p_pool = ctx.enter_context(tc.tile_pool(name="h1p", bufs=1))
            h2_pool = ctx.enter_context(tc.tile_pool(name="h2", bufs=1))
            x_pool = ctx.enter_context(tc.tile_pool(name="x", bufs=2))
            o_pool = ctx.enter_context(tc.tile_pool(name="o", bufs=2))
            t3_pool = ctx.enter_context(tc.tile_pool(name="t3", bufs=3))
            tmp_pool = ctx.enter_context(tc.tile_pool(name="tmp", bufs=3))
            pa_pool = ctx.enter_context(tc.tile_pool(name="pa", bufs=2, space="PSUM"))
            pb_pool = ctx.enter_context(tc.tile_pool(name="pb", bufs=2, space="PSUM"))
            pc_pool = ctx.enter_context(tc.tile_pool(name="pc", bufs=2, space="PSUM"))

            w1f_sb = consts.tile([66, HID], fp32)
            nc.sync.dma_start(w1f_sb[:], w1f)
            wdw_sb = consts.tile([128, NG * 9 * 128], fp32)
            nc.sync.dma_start(wdw_sb[:], wdw)
            b2m_sb = consts.tile([128, NG], fp32)
            nc.sync.dma_start(b2m_sb[:], b2m)
            w3f_sb = consts.tile([128, NG * 64], fp32)
            nc.sync.dma_start(w3f_sb[:], w3f)
            b3m_sb = consts.tile([64, 1], fp32)
            nc.sync.dma_start(b3m_sb[:], b3m)
            negm_sb = consts.tile([128, 1], fp32)
            nc.gpsimd.memset(negm_sb[:], -float(MAGIC))

            # persistent padded H1 (integer grid) per channel group; zero
            # borders written once, interior overwritten every image.
            h1p = [h1p_pool.tile([128, PPIX], fp32, tag=f"h1p{g}",
                                 name=f"h1p{g}") for g in range(NG)]
            for g in range(NG):
                nc.gpsimd.memset(h1p[g][:], 0.0)
            h1v = [t[:].rearrange("p (h w) -> p h w", w=PW) for t in h1p]
            h2 = [h2_pool.tile([128, PIX], fp32, tag=f"h2{g}",
                               name=f"h2{g}") for g in range(NG)]

            for i in range(BPC):
                x_sb = x_pool.tile([66, PIX], fp32)
                nc.sync.dma_start(x_sb[:], xs[i, :, :])
                o_sb = o_pool.tile([64, PIX], fp32)

                # ---- stage 1: expand conv + quant ----
                for g in range(NG):
                    for t in range(NT):
                        pa = pa_pool.tile([128, TW], fp32)
                        nc.tensor.matmul(
                            pa[:],
                            w1f_sb[:, 128 * g:128 * (g + 1)],
                            x_sb[:, TW * t:TW * (t + 1)],
                            start=True, stop=True)
                        r0 = ROWS_PT * t + 1
                        dst = h1v[g][:, r0:r0 + ROWS_PT, 1:57]
                        # round via fp32 magic-add on ACT (PE accumulation
                        # order is not sequential, so PSUM can't carry 2^23)
                        nc.scalar.activation(dst, pa[:], COPY,
                                             bias=float(MAGIC), scale=1.0)
                        nc.vector.tensor_scalar(dst, dst,
                                                float(MAGIC), 0.0,
                                                op0=sub, op1=mx)
                        nc.vector.tensor_scalar_min(dst, dst, 255.0)

                # ---- stage 2: depthwise 3x3 + quant ----
                for g in range(NG):
                    for t in range(NT):
                        pb = pb_pool.tile([128, TW], fp32)
                        r0 = ROWS_PT * t + 1
                        k = 0
                        for dy in (-1, 0, 1):
                            for dx in (-1, 0, 1):
                                rhs = h1v[g][:, r0 + dy:r0 + dy + ROWS_PT,
                                             1 + dx:57 + dx]
                                lcol = 128 * (9 * g + k)
                                nc.tensor.matmul(
                                    pb[:], wdw_sb[:, lcol:lcol + 128], rhs,
                                    start=(k == 0), stop=(k == 8))
                                k += 1
                        tmp = tmp_pool.tile([128, TW], fp32)
                        nc.scalar.activation(tmp[:], pb[:], IDENT,
                                             bias=b2m_sb[:, g:g + 1], scale=1.0)
                        nc.vector.tensor_scalar(tmp[:], tmp[:],
                                                float(MAGIC), float(MAGIC),
                                                op0=add, op1=mx)
                        nc.vector.tensor_scalar(h2[g][:, TW * t:TW * (t + 1)],
                                                tmp[:],
                                                float(MAGIC) + 255.0, float(MAGIC),
                                                op0=mn, op1=sub)

                # ---- stage 3: project conv + quant + residual ----
                for t in range(NT):
                    pc = pc_pool.tile([64, TW], fp32)
                    for kc in range(NG):
                        nc.tensor.matmul(
                            pc[:], w3f_sb[:, 64 * kc:64 * (kc + 1)],
                            h2[kc][:, TW * t:TW * (t + 1)],
                            start=(kc == 0), stop=(kc == NG - 1))
                    t3 = t3_pool.tile([64, TW], fp32)
                    nc.scalar.activation(t3[:], pc[:], IDENT,
                                         bias=b3m_sb[:, 0:1], scale=1.0)
                    nc.vector.tensor_scalar(t3[:], t3[:],
                                            float(MAGIC), float(MAGIC),
                                            op0=add, op1=mx)
                    nc.vector.tensor_scalar(t3[:], t3[:],
                                            float(MAGIC) + 255.0, float(MAGIC),
                                            op0=mn, op1=sub)
                    nc.vector.scalar_tensor_tensor(
                        o_sb[:, TW * t:TW * (t + 1)], t3[:],
                        float(np.float32(1.0 / 255.0)),
                        x_sb[0:64, TW * t:TW * (t + 1)],
                        op0=mult, op1=add)

                nc.sync.dma_start(ys[i, :, :], o_sb[:])

    nc.compile()
    return nc


def _prep_weights(inputs):
    inv1 = (inputs['g1'] / np.sqrt(inputs['v1'] + EPS)).astype(np.float32)
    beta1 = (inputs['b1'] - inputs['m1'] * inv1).astype(np.float32)
    inv2 = (inputs['g2'] / np.sqrt(inputs['v2'] + EPS)).astype(np.float32)
    beta2 = (inputs['b2'] - inputs['m2'] * inv2).astype(np.float32)
    inv3 = (inputs['g3'] / np.sqrt(inputs['v3'] + EPS)).astype(np.float32)
    beta3 = (inputs['b3'] - inputs['m3'] * inv3).astype(np.float32)

    w1q = _quant_w(inputs['w1'])[:, :, 0, 0]       # [384, 64]
    w2q = _quant_w(inputs['w2'])[:, 0, :, :]       # [384, 3, 3]
    w3q = _quant_w(inputs['w3'])[:, :, 0, 0]       # [64, 384]

    w1f = np.zeros((66, HID), np.float32)
    w1f[0:64, :] = (w1q * (inv1 * np.float32(42.5))[:, None]).T
    w1f[64, :] = beta1 * np.float32(42.5)
    # row 65 stays zero: magic rounding happens post-PE (PSUM accumulation
    # order is not strictly sequential, so +2^23 inside the matmul mis-rounds)

    wdw = np.zeros((128, NG * 9 * 128), np.float32)
    for g in range(NG):
        ch = slice(128 * g, 128 * (g + 1))
        # H1 is stored as integers r1 = H1q*(255/6); output needs bn*42.5,
        # so the diag weight is w2q * (6/255) * inv2 * 42.5 = w2q * inv2.
        scale = (w2q[ch] * inv2[ch][:, None, None])
        k = 0
        for dy in range(3):
            for dx in range(3):
                col = 128 * (9 * g + k)
                wdw[:, col:col + 128][np.arange(128), np.arange(128)] = scale[:, dy, dx]
                k += 1

    b2m = (beta2 * np.float32(42.5)).reshape(NG, 128).T.copy()   # [128, NG]

    w3f = np.zeros((128, NG * 64), np.float32)
    w3full = (w3q * (np.float32(6.0) * inv3)[:, None])           # [64, 384]
    for kc in range(NG):
        w3f[:, 64 * kc:64 * (kc + 1)] = w3full[:, 128 * kc:128 * (kc + 1)].T

    b3m = (beta3 * np.float32(255.0)).reshape(64, 1)
    return w1f, np.ascontiguousarray(wdw), np.ascontiguousarray(b2m), w3f, b3m


def kernel(**inputs):
    from concourse import bass_utils

    if 'nc' not in _cache:
        _cache['nc'] = _build_program()
    nc = _cache['nc']

    w1f, wdw, b2m, w3f, b3m = _prep_weights(inputs)
    x = np.asarray(inputs['x'], np.float32).reshape(B, C, PIX)

    in_maps = []
    for c in range(NCORES):
        xs = np.ones((BPC, 66, PIX), np.float32)
        xs[:, 0:64, :] = x[BPC * c:BPC * (c + 1)]
        in_maps.append({'xs': xs, 'w1f': w1f, 'wdw': wdw, 'b2m': b2m,
                        'w3f': w3f, 'b3m': b3m})

    res = bass_utils.run_bass_kernel_spmd(nc, in_maps, list(range(NCORES)))
    out = np.concatenate([res.results[c]['ys'] for c in range(NCORES)], axis=0)
    return out.reshape(B, C, H, W).astype(np.float32)


# revision 18
# speedup vs baseline: 2.3374x; 2.3374x over previous
"""Trainium2 Bass kernel for quantized InvertedResidual block (DoReFa fake-quant).

Strategy:
- Data-parallel: 32 images -> 4 per core across 8 NeuronCores.
- Host precomputes DoReFa-quantized weights, folds BN scale (and the
  42.5 / 255 quant scales) into matmul weights, and appends bias +
  2^23 magic-rounding rows to the conv1 weight matrix.
- Device per image:
  stage1: 1x1 conv (K=66 incl. bias+magic rows) -> PSUM holds
          round_int(bn(y)*42.5)+2^23; ACT Relu(x-2^23) -> integers,
          DVE min 255 -> r1 in 0..255 stored f32 in a zero-padded
          [128, 58*58] SBUF layout per 128-channel group.
  stage2: depthwise 3x3 = 9 diagonal matmuls (weights w2q*inv2*42.5)
          accumulating in PSUM over shifted 2D access patterns;
          ACT Identity(+beta2*42.5), DVE (add 2^23, max 2^23),
          DVE (min 2^23+255, sub 2^23) -> r2 integers.
  stage3: 1x1 conv over r2 (weights w3q*6*inv3), ACT Identity(+beta3*255),
          two DVE clamps, then (r3/255) + x residual via scalar_tensor_tensor.
All arithmetic fp32; activations kept on the exact 8-bit integer grid.
"""
import numpy as np

EPS = 1e-5
MAGIC = np.float32(2.0 ** 23)

B, C, H, W = 32, 64, 56, 56
HID = 384
NCORES = 8
BPC = B // NCORES          # images per core
PIX = H * W                # 3136
PW = W + 2                 # 58
PH = H + 2
PPIX = PW * PH             # 3364
NT = 7                     # pixel tiles per image
TW = PIX // NT             # 448 = 8 rows x 56
ROWS_PT = H // NT          # 8
NG = HID // 128            # 3 channel groups

_cache = {}


def _quant_w(w):
    # DoReFa weight fake-quant, computed with jax on CPU so tanh/round are
    # bitwise identical to the reference implementation.
    import jax
    import jax.numpy as jnp
    with jax.default_device(jax.devices('cpu')[0]):
        t = jnp.tanh(jnp.asarray(w, jnp.float32))
        m = jnp.max(jnp.abs(t), axis=(1, 2, 3), keepdims=True)
        wn = t / (2.0 * m) + 0.5
        q = 2.0 * jnp.round(wn * 255.0) / 255.0 - 1.0
        return np.asarray(q, np.float32)


def _build_program():
    import concourse.bass as bass
    import concourse.tile as tile
    from concourse import bacc, mybir

    fp32 = mybir.dt.float32
    bf16 = mybir.dt.bfloat16
    nc = bacc.Bacc("TRN2", target_bir_lowering=False, debug=False,
                   enable_asserts=False, num_devices=NCORES)

    xs = nc.dram_tensor("xs", [BPC, 66, PIX], fp32, kind="ExternalInput").ap()
    w1f = nc.dram_tensor("w1f", [66, HID], fp32, kind="ExternalInput").ap()
    # depthwise diag weights as exact small integers (2j-255) in bf16
    wdw = nc.dram_tensor("wdw", [128, NG * 9 * 128], bf16, kind="ExternalInput").ap()
    b2m = nc.dram_tensor("b2m", [128, NG], fp32, kind="ExternalInput").ap()
    s2m = nc.dram_tensor("s2m", [128, NG], fp32, kind="ExternalInput").ap()
    w3f = nc.dram_tensor("w3f", [128, NG * 64], fp32, kind="ExternalInput").ap()
    b3m = nc.dram_tensor("b3m", [64, 1], fp32, kind="ExternalInput").ap()
    ys = nc.dram_tensor("ys", [BPC, 64, PIX], fp32, kind="ExternalOutput").ap()

    mx = mybir.AluOpType.max
    mn = mybir.AluOpType.min
    add = mybir.AluOpType.add
    sub = mybir.AluOpType.subtract
    mult = mybir.AluOpType.mult
    RELU = mybir.ActivationFunctionType.Relu  # noqa: F841
    COPY = mybir.ActivationFunctionType.Copy
    IDENT = mybir.ActivationFunctionType.Identity

    with tile.TileContext(nc) as tc:
        from contextlib import ExitStack
        with ExitStack() as ctx:
            consts = ctx.enter_context(tc.tile_pool(name="consts", bufs=1))
            h1p_pool = ctx.enter_context(tc.tile_pool(name="h1p", bufs=1))
            h2_pool = ctx.enter_context(tc.tile_pool(name="h2", bufs=1))
            x_pool = ctx.enter_context(tc.tile_pool(name="x", bufs=2))
            o_pool = ctx.enter_context(tc.tile_pool(name="o", bufs=2))
            t3_pool = ctx.enter_context(tc.tile_pool(name="t3", bufs=3))
            tmp_pool = ctx.enter_context(tc.tile_pool(name="tmp", bufs=3))
            pa_pool = ctx.enter_context(tc.tile_pool(name="pa", bufs=2, space="PSUM"))
            pb_pool = ctx.enter_context(tc.tile_pool(name="pb", bufs=2, space="PSUM"))
            pc_pool = ctx.enter_context(tc.tile_pool(name="pc", bufs=2, space="PSUM"))

            w1f_sb = consts.tile([66, HID], fp32)
            nc.sync.dma_start(w1f_sb[:], w1f)
            wdw_sb = consts.tile([128, NG * 9 * 128], bf16)
            nc.sync.dma_start(wdw_sb[:], wdw)
            b2m_sb = consts.tile([128, NG], fp32)
            nc.sync.dma_start(b2m_sb[:], b2m)
            s2m_sb = consts.tile([128, NG], fp32)
            nc.sync.dma_start(s2m_sb[:], s2m)
            w3f_sb = consts.tile([128, NG * 64], fp32)
            nc.sync.dma_start(w3f_sb[:], w3f)
            b3m_sb = consts.tile([64, 1], fp32)
            nc.sync.dma_start(b3m_sb[:], b3m)
            negm_sb = consts.tile([128, 1], fp32)
            nc.gpsimd.memset(negm_sb[:], -float(MAGIC))

            # persistent padded H1 (integer grid) per channel group; zero
            # borders written once, interior overwritten every image.
            h1p = [h1p_pool.tile([128, PPIX], bf16, tag=f"h1p{g}",
                                 name=f"h1p{g}") for g in range(NG)]
            for g in range(NG):
                nc.gpsimd.memset(h1p[g][:], 0.0)
            h1v = [t[:].rearrange("p (h w) -> p h w", w=PW) for t in h1p]
            h2 = [h2_pool.tile([128, PIX], fp32, tag=f"h2{g}",
                               name=f"h2{g}") for g in range(NG)]

            for i in range(BPC):
                x_sb = x_pool.tile([66, PIX], fp32)
                nc.sync.dma_start(x_sb[:], xs[i, :, :])
                o_sb = o_pool.tile([64, PIX], fp32)

                # ---- stage 1: expand conv + quant ----
                for g in range(NG):
                    for t in range(NT):
                        pa = pa_pool.tile([128, TW], fp32)
                        nc.tensor.matmul(
                            pa[:],
                            w1f_sb[:, 128 * g:128 * (g + 1)],
                            x_sb[:, TW * t:TW * (t + 1)],
                            start=True, stop=True)
                        r0 = ROWS_PT * t + 1
                        dst = h1v[g][:, r0:r0 + ROWS_PT, 1:57]
                        # round via fp32 magic-add on ACT (PE accumulation
                        # order is not sequential, so PSUM can't carry 2^23);
                        # final integers 0..255 are bf16-exact.
                        tmp1 = tmp_pool.tile([128, TW], fp32, tag="tmp1",
                                             name="tmp1")
                        nc.scalar.activation(tmp1[:], pa[:], COPY,
                                             bias=float(MAGIC), scale=1.0)
                        nc.vector.tensor_scalar(tmp1[:], tmp1[:],
                                                float(MAGIC), 0.0,
                                                op0=sub, op1=mx)
                        nc.vector.tensor_scalar(dst, tmp1[:], 255.0, None,
                                                op0=mn)

                # ---- stage 2: depthwise 3x3 + quant ----
                for g in range(NG):
                    for t in range(NT):
                        pb = pb_pool.tile([128, TW], fp32)
                        r0 = ROWS_PT * t + 1
                        k = 0
                        for dy in (-1, 0, 1):
                            for dx in (-1, 0, 1):
                                rhs = h1v[g][:, r0 + dy:r0 + dy + ROWS_PT,
                                             1 + dx:57 + dx]
                                lcol = 128 * (9 * g + k)
                                nc.tensor.matmul(
                                    pb[:], wdw_sb[:, lcol:lcol + 128], rhs,
                                    start=(k == 0), stop=(k == 8))
                                k += 1
                        tmp = tmp_pool.tile([128, TW], fp32)
                        nc.scalar.activation(tmp[:], pb[:], IDENT,
                                             bias=b2m_sb[:, g:g + 1],
                                             scale=s2m_sb[:, g:g + 1])
                        nc.vector.tensor_scalar(tmp[:], tmp[:],
                                                float(MAGIC), float(MAGIC),
                                                op0=add, op1=mx)
                        nc.vector.tensor_scalar(h2[g][:, TW * t:TW * (t + 1)],
                                                tmp[:],
                                                float(MAGIC) + 255.0, float(MAGIC),
                                                op0=mn, op1=sub)

                # ---- stage 3: project conv + quant + residual ----
                for t in range(NT):
                    pc = pc_pool.tile([64, TW], fp32)
                    for kc in range(NG):
                        nc.tensor.matmul(
                            pc[:], w3f_sb[:, 64 * kc:64 * (kc + 1)],
                            h2[kc][:, TW * t:TW * (t + 1)],
                            start=(kc == 0), stop=(kc == NG - 1))
                    t3 = t3_pool.tile([64, TW], fp32)
                    nc.scalar.activation(t3[:], pc[:], IDENT,
                                         bias=b3m_sb[:, 0:1], scale=1.0)
                    nc.vector.tensor_scalar(t3[:], t3[:],
                                            float(MAGIC), float(MAGIC),
                                            op0=add, op1=mx)
                    nc.vector.tensor_scalar(t3[:], t3[:],
                                            float(MAGIC) + 255.0, float(MAGIC),
                                            op0=mn, op1=sub)
                    nc.vector.scalar_tensor_tensor(
                        o_sb[:, TW * t:TW * (t + 1)], t3[:],
                        float(np.float32(1.0 / 255.0)),
                        x_sb[0:64, TW * t:TW * (t + 1)],
                        op0=mult, op1=add)

                nc.sync.dma_start(ys[i, :, :], o_sb[:])

    nc.compile()
    return nc


def _prep_weights(inputs):
    inv1 = (inputs['g1'] / np.sqrt(inputs['v1'] + EPS)).astype(np.float32)
    beta1 = (inputs['b1'] - inputs['m1'] * inv1).astype(np.float32)
    inv2 = (inputs['g2'] / np.sqrt(inputs['v2'] + EPS)).astype(np.float32)
    beta2 = (inputs['b2'] - inputs['m2'] * inv2).astype(np.float32)
    inv3 = (inputs['g3'] / np.sqrt(inputs['v3'] + EPS)).astype(np.float32)
    beta3 = (inputs['b3'] - inputs['m3'] * inv3).astype(np.float32)

    w1q = _quant_w(inputs['w1'])[:, :, 0, 0]       # [384, 64]
    w2q = _quant_w(inputs['w2'])[:, 0, :, :]       # [384, 3, 3]
    w3q = _quant_w(inputs['w3'])[:, :, 0, 0]       # [64, 384]

    w1f = np.zeros((66, HID), np.float32)
    w1f[0:64, :] = (w1q * (inv1 * np.float32(42.5))[:, None]).T
    w1f[64, :] = beta1 * np.float32(42.5)
    # row 65 stays zero: magic rounding happens post-PE (PSUM accumulation
    # order is not strictly sequential, so +2^23 inside the matmul mis-rounds)

    import ml_dtypes
    # exact integer depthwise weights: w2q = (2j-255)/255, so w2q*255 rounds
    # to the exact integer level; bf16 holds |ints| <= 255 exactly. The
    # (1/255)*inv2 factor (and H1's 6/255, folded: *42.5*6/255 = inv2/255)
    # is applied by the stage-2 ACT scale.
    wdw_int = np.round(w2q * np.float32(255.0))
    wdw = np.zeros((128, NG * 9 * 128), ml_dtypes.bfloat16)
    for g in range(NG):
        ch = slice(128 * g, 128 * (g + 1))
        k = 0
        for dy in range(3):
            for dx in range(3):
                col = 128 * (9 * g + k)
                wdw[:, col:col + 128][np.arange(128), np.arange(128)] = \
                    wdw_int[ch, dy, dx].astype(ml_dtypes.bfloat16)
                k += 1

    b2m = (beta2 * np.float32(42.5)).reshape(NG, 128).T.copy()   # [128, NG]
    s2m = (inv2 / np.float32(255.0)).reshape(NG, 128).T.copy()   # [128, NG]

    w3f = np.zeros((128, NG * 64), np.float32)
    w3full = (w3q * (np.float32(6.0) * inv3)[:, None])           # [64, 384]
    for kc in range(NG):
        w3f[:, 64 * kc:64 * (kc + 1)] = w3full[:, 128 * kc:128 * (kc + 1)].T

    b3m = (beta3 * np.float32(255.0)).reshape(64, 1)
    return (w1f, np.ascontiguousarray(wdw), np.ascontiguousarray(b2m),
            np.ascontiguousarray(s2m), w3f, b3m)


def kernel(**inputs):
    from concourse import bass_utils

    if 'nc' not in _cache:
        _cache['nc'] = _build_program()
    nc = _cache['nc']

    w1f, wdw, b2m, s2m, w3f, b3m = _prep_weights(inputs)
    x = np.asarray(inputs['x'], np.float32).reshape(B, C, PIX)

    in_maps = []
    for c in range(NCORES):
        xs = np.ones((BPC, 66, PIX), np.float32)
        xs[:, 0:64, :] = x[BPC * c:BPC * (c + 1)]
        in_maps.append({'xs': xs, 'w1f': w1f, 'wdw': wdw, 'b2m': b2m,
                        's2m': s2m, 'w3f': w3f, 'b3m': b3m})

    res = bass_utils.run_bass_kernel_spmd(nc, in_maps, list(range(NCORES)))
    out = np.concatenate([res.results[c]['ys'] for c in range(NCORES)], axis=0)
    return out.reshape(B, C, H, W).astype(np.float32)
